# revision 1
# baseline (speedup 1.0000x reference)
"""Multi-head attention (b=4, n=2048, d=768, h=12) on 8 trn2 NeuronCores.

Sharding: (batch x sequence-half) -> 8 shards. Each core gets one batch's
x (rotated by half the sequence for odd cores, which is legal because
attention is permutation-invariant over key positions), computes K/V for
the full sequence and Q for the first 1024 rows, and returns those 1024
output rows. Host concatenates. No collectives needed.

Device algorithm (per core); storage fp32, matmul operands float32r
(TF32-like relaxed fp32: 4x faster on the PE than strict fp32, rel err
~4e-4 end to end):
  1. x^T supplied pre-transposed by the host, DMAd straight to SBUF.
  2. Per head pair: K^T/Q^T/V^T = (x @ W)^T via lhsT=W chunks; V^T is
     PE-transposed back to natural [n, d] layout (+ ones column per head
     for the softmax denominators).
  3. Per head: S^T[k,q] = K^T_slice.T @ Q^T (no P transposes needed),
     P^T = exp(S^T * scale) on ACT, O^T[65,1024] += V_aug.T @ P^T.
     Row 64 of O^T = softmax denominators.
  4. Per (head, qblock): projection is applied per head with the
     normalization folded in: denominators land on partitions via a K=1
     matmul of O^T row 64, then acc[qb] += r_q * (O_un,h @ W_h) using a
     fused scalar_tensor_tensor. acc is seeded with the bias, so phase D
     is just 8 output DMAs -- no transposes, no serial projection tail.
"""

import numpy as np

B, N, D, H, HD = 4, 2048, 768, 12, 64
NQ = N // 2            # query rows per core
SCALE = HD ** -0.5
NCORES = 8
NT = N // 128          # 16 key tiles
DT = D // 128          # 6 d-chunks
QT = NQ // 128         # 8 query blocks
NPAIR = H // 2         # 6 head pairs

_RUNNER = None


def _build_program(reps=1):
    import concourse.bass as bass
    import concourse.tile as tile
    import concourse.mybir as mybir
    from concourse import bacc
    from concourse.masks import make_identity
    from contextlib import ExitStack

    f32 = mybir.dt.float32
    AF = mybir.ActivationFunctionType
    ALU = mybir.AluOpType
    f32r = mybir.dt.float32r

    nc = bacc.Bacc("TRN2", target_bir_lowering=False, debug=False,
                   num_devices=NCORES)

    xt_in = nc.dram_tensor("xt", [D, N], f32, kind="ExternalInput")
    w_qkv = nc.dram_tensor("w_qkv", [D, 3 * D], f32, kind="ExternalInput")
    w_proj = nc.dram_tensor("w_proj", [D, D], f32, kind="ExternalInput")
    b_proj = nc.dram_tensor("b_proj", [D], f32, kind="ExternalInput")
    y = nc.dram_tensor("y", [NQ, D], f32, kind="ExternalOutput")

    with tile.TileContext(nc) as tc:
      for _rep in range(reps):
        with ExitStack() as ctx:
          singles = ctx.enter_context(tc.tile_pool(name="singles", bufs=1))
          onat_pool = ctx.enter_context(tc.tile_pool(name="onat", bufs=1))
          small_pool = ctx.enter_context(tc.tile_pool(name="small", bufs=4))
          # psum pool for transposes / QKV accumulation / misc (2 banks)
          ps_misc = ctx.enter_context(
              tc.tile_pool(name="ps_misc", bufs=2, space="PSUM"))

          ident = singles.tile([128, 128], f32)
          make_identity(nc, ident)

          bias_bc = singles.tile([128, D], f32)
          b_ap = b_proj.ap()
          b_bcast = bass.AP(tensor=b_ap.tensor, offset=b_ap.offset,
                            ap=[[0, 128]] + list(b_ap.ap))
          nc.gpsimd.dma_start(out=bias_bc[:], in_=b_bcast)

          ones128 = singles.tile([128, 2], f32r)
          nc.vector.memset(ones128[:].bitcast(f32), 1.0)

          # warm the ACT exp table set during the startup DMA window so the
          # ~2.7us ACT_TABLE_LOAD+drain is off the first real exp's path
          warm = small_pool.tile([128, 1], f32, tag="warm", name="warm")
          nc.vector.memset(warm[:], 0.0)
          nc.scalar.activation(warm[:], warm[:], AF.Exp, bias=0.0, scale=1.0)

          # w_proj pair-row tiles [128, D], resident for the whole kernel
          wp = []
          for p in range(NPAIR):
              t = singles.tile([128, D], f32r, tag=f"wp{p}", name=f"wp{p}")
              deng = nc.sync if p % 2 == 0 else nc.gpsimd
              deng.dma_start(
                  out=t[:],
                  in_=w_proj[p * 128:(p + 1) * 128, :].bitcast(f32r))
              wp.append(t)

          # per-qblock output accumulators [128, 768], seeded with the bias;
          # each head adds r_q * (O_un,h @ W_h) during its attention phase
          acc = [onat_pool.tile([128, D], f32, tag=f"acc{qb}", name=f"acc{qb}")
                 for qb in range(QT)]
          for qb in range(QT):
              nc.vector.tensor_copy(out=acc[qb][:], in_=bias_bc[:])

          with tc.tile_pool(name="xt", bufs=1) as xt_pool:
              # ---- Phase A: load pre-transposed x^T directly ----
              xT = [xt_pool.tile([128, N], f32r, tag=f"xt{dt}", name=f"xt{dt}")
                    for dt in range(DT)]

              # ---- Phase C: per head pair: K^T, Q^T, V; then attention ----
              with tc.tile_pool(name="wkq", bufs=16) as wkq_pool, \
                   tc.tile_pool(name="kq", bufs=2) as kq_pool, \
                   tc.tile_pool(name="vsb", bufs=2) as vsb_pool, \
                   tc.tile_pool(name="pt", bufs=3) as pt_pool, \
                   tc.tile_pool(name="otsb", bufs=2) as ot_pool, \
                   tc.tile_pool(name="ps_st", bufs=2, space="PSUM") as ps_st, \
                   tc.tile_pool(name="ps_ot", bufs=1, space="PSUM") as ps_ot:
                  def emit_w(p):
                      wk, wq, wv = [], [], []
                      for dt in range(DT):
                          for lst, tag, col0 in ((wk, "wk", D + p * 128),
                                                 (wq, "wq", p * 128),
                                                 (wv, "wv", 2 * D + p * 128)):
                              t = wkq_pool.tile([128, 128], f32r, tag=tag,
                                                name=tag)
                              deng = nc.sync if dt % 2 == 0 else nc.gpsimd
                              deng.dma_start(
                                  out=t[:],
                                  in_=w_qkv[dt * 128:(dt + 1) * 128,
                                            col0:col0 + 128].bitcast(f32r))
                              lst.append(t)
                      return wk, wq, wv

                  def emit_qkv_a(p, w3):
                      wk, wq, wv = w3

                      # K^T_pair [128, 2048] in halves (2 psum slices at a time)
                      KT = kq_pool.tile([128, N], f32r, tag="kt")
                      for half in range(2):
                          pss = [ps_misc.tile([128, 512], f32, tag="misc", name="pss")
                                 for _ in range(2)]
                          for dt in range(DT):
                              for i in range(2):
                                  ns = half * 2 + i
                                  nc.tensor.matmul(
                                      pss[i][:], lhsT=wk[dt][:],
                                      rhs=xT[dt][:, ns * 512:(ns + 1) * 512],
                                      start=(dt == 0), stop=(dt == DT - 1))
                          for i in range(2):
                              ns = half * 2 + i
                              nc.vector.tensor_copy(
                                  out=KT[:, ns * 512:(ns + 1) * 512],
                                  in_=pss[i][:])

                      # Q^T_pair [128, 1024]
                      QTt = kq_pool.tile([128, NQ], f32r, tag="qt")
                      pss = [ps_misc.tile([128, 512], f32, tag="misc", name="pss")
                             for _ in range(2)]
                      for dt in range(DT):
                          for i in range(2):
                              nc.tensor.matmul(
                                  pss[i][:], lhsT=wq[dt][:],
                                  rhs=xT[dt][:, i * 512:(i + 1) * 512],
                                  start=(dt == 0), stop=(dt == DT - 1))
                      for i in range(2):
                          nc.vector.tensor_copy(
                              out=QTt[:, i * 512:(i + 1) * 512], in_=pss[i][:])

                      return KT, QTt, wv

                  def emit_qkv_b(p, wv):
                      # V^T_pair [128, 2048], then transpose to natural V_sb
                      # [128, nt, 130]; cols per pair:
                      # [headA 64 | ones | headB 64 | ones]
                      VTt = kq_pool.tile([128, N], f32, tag="vt")
                      for half in range(2):
                          pss = [ps_misc.tile([128, 512], f32, tag="misc", name="pss")
                                 for _ in range(2)]
                          for dt in range(DT):
                              for i in range(2):
                                  ns = half * 2 + i
                                  nc.tensor.matmul(
                                      pss[i][:], lhsT=wv[dt][:],
                                      rhs=xT[dt][:, ns * 512:(ns + 1) * 512],
                                      start=(dt == 0), stop=(dt == DT - 1))
                          for i in range(2):
                              ns = half * 2 + i
                              nc.vector.tensor_copy(
                                  out=VTt[:, ns * 512:(ns + 1) * 512],
                                  in_=pss[i][:])

                      Vsb = vsb_pool.tile([128, NT, 130], f32r, tag="v")
                      nc.vector.memset(Vsb[:, :, 64:65].bitcast(f32), 1.0)
                      nc.vector.memset(Vsb[:, :, 129:130].bitcast(f32), 1.0)
                      for kt in range(NT):
                          ps = ps_misc.tile([128, 128], f32, tag="misc")
                          nc.tensor.transpose(
                              ps[:], VTt[:, kt * 128:(kt + 1) * 128], ident[:])
                          dst = Vsb[:, kt, :].rearrange("n (h c) -> n h c", h=2)
                          nc.vector.tensor_copy(
                              out=dst[:, :, 0:64],
                              in_=ps[:].rearrange("n (h c) -> n h c", h=2))
                      return Vsb

                  def emit_attn_head(p, hh, KT, QTt, Vsb):
                          h = 2 * p + hh
                          base = hh * 64
                          ot = ps_ot.tile([65, NQ], f32, tag="ot")
                          for kt in range(NT):
                              st = ps_st.tile([128, NQ], f32, tag="st")
                              lhsT = KT[base:base + 64, kt * 128:(kt + 1) * 128]
                              for i in range(2):
                                  nc.tensor.matmul(
                                      st[:, i * 512:(i + 1) * 512],
                                      lhsT=lhsT,
                                      rhs=QTt[base:base + 64,
                                              i * 512:(i + 1) * 512],
                                      start=True, stop=True)
                              pt = pt_pool.tile([128, NQ], f32r, tag="pt")
                              nc.scalar.activation(pt[:], st[:], AF.Exp,
                                                   bias=0.0, scale=float(SCALE))
                              vh = Vsb[:, kt, hh * 65: hh * 65 + 65]
                              for i in range(2):
                                  nc.tensor.matmul(
                                      ot[:, i * 512:(i + 1) * 512], lhsT=vh,
                                      rhs=pt[:, i * 512:(i + 1) * 512],
                                      start=(kt == 0), stop=(kt == NT - 1))

                          # head A: O rows at partitions 0-63, denom at 64
                          # head B: O rows at partitions 64-127, denom at 0 --
                          # so lhsT/rhs base partitions match for the proj
                          # matmuls (lhsT base == rhs base is required)
                          otsb = ot_pool.tile([128, NQ], f32r, tag="otsb")
                          if hh == 0:
                              nc.vector.tensor_copy(out=otsb[0:65, :],
                                                    in_=ot[:])
                              orows = otsb[0:64, :]
                              drow = otsb[64:65, :]
                              done = ones128[64:65, :]
                          else:
                              nc.vector.tensor_copy(out=otsb[64:128, :],
                                                    in_=ot[0:64, :])
                              nc.vector.tensor_copy(out=otsb[0:1, :],
                                                    in_=ot[64:65, :])
                              orows = otsb[64:128, :]
                              drow = otsb[0:1, :]
                              done = ones128[0:1, :]
                          wph = wp[p][hh * 64:(hh + 1) * 64, :]
                          def emit_proj(orows=orows, drow=drow, done=done,
                                        wph=wph):
                            for qb in range(QT):
                                qsl = slice(qb * 128, (qb + 1) * 128)
                                # denominators onto partitions via a K=1 matmul
                                sden = ps_misc.tile([128, 2], f32, tag="misc",
                                                    name="sden")
                                nc.tensor.matmul(sden[:], lhsT=drow[:, qsl],
                                                 rhs=done,
                                                 start=True, stop=True)
                                rcp = small_pool.tile([128, 1], f32, tag="rcp")
                                nc.vector.reciprocal(rcp[:], sden[:, 0:1])
                                for i in range(2):
                                    pp = ps_misc.tile([128, 384], f32,
                                                      tag="misc", name="pp")
                                    nc.tensor.matmul(
                                        pp[:], lhsT=orows[:, qsl],
                                        rhs=wph[:, i * 384:(i + 1) * 384],
                                        start=True, stop=True)
                                    nc.vector.scalar_tensor_tensor(
                                        out=acc[qb][:, i * 384:(i + 1) * 384],
                                        in0=pp[:], scalar=rcp[:],
                                        in1=acc[qb][:, i * 384:(i + 1) * 384],
                                        op0=ALU.mult, op1=ALU.add)
                          return emit_proj

                  # software pipeline: QKV of pair p+1 is emitted
                  # interleaved with the attention heads of pair p so PE
                  # fills ACT-bound stretches without starving ACT at the
                  # pair boundary
                  # pair-0 weight DMAs first (tiny), then x^T in
                  # interleaved 512-col slices across both queues so the
                  # first QKV matmuls can start within a few us
                  w3_first = emit_w(0)
                  for ns in range(4):
                      for dt in range(DT):
                          deng = (nc.sync, nc.gpsimd,
                                  nc.scalar, nc.sync)[ns]
                          deng.dma_start(
                              out=xT[dt][:, ns * 512:(ns + 1) * 512],
                              in_=xt_in[dt * 128:(dt + 1) * 128,
                                        ns * 512:(ns + 1) * 512].bitcast(f32r))
                  kt0, qt0, wv0 = emit_qkv_a(0, w3_first)
                  vs0 = emit_qkv_b(0, wv0)
                  cur = (kt0, qt0, vs0)
                  pending_proj = None
                  for p in range(NPAIR):
                      if p + 1 < NPAIR:
                          nxt_a = emit_qkv_a(p + 1, emit_w(p + 1))
                          nxt = (nxt_a[0], nxt_a[1],
                                 emit_qkv_b(p + 1, nxt_a[2]))
                      else:
                          nxt = None
                      pj = emit_attn_head(p, 0, *cur)
                      if pending_proj is not None:
                          pending_proj()
                      pending_proj = pj
                      pj = emit_attn_head(p, 1, *cur)
                      pending_proj()
                      pending_proj = pj
                      cur = nxt
                  pending_proj()

          # ---- Phase D: write the accumulated outputs ----
          for qb in range(QT):
              deng = nc.sync if qb % 2 == 0 else nc.gpsimd
              deng.dma_start(out=y[qb * 128:(qb + 1) * 128, :],
                             in_=acc[qb][:])

    nc.compile()
    return nc


def _make_runner(nc):
    """Cached multi-core PJRT runner (mirrors run_bass_via_pjrt, but the
    jitted callable is built once and reused across kernel() calls)."""
    import jax
    from jax.experimental.shard_map import shard_map
    from jax.sharding import Mesh, PartitionSpec
    import concourse.mybir as mybir
    from concourse.bass2jax import (_bass_exec_p, install_neuronx_cc_hook,
                                    partition_id_tensor)

    install_neuronx_cc_hook()

    partition_name = (nc.partition_id_tensor.name
                      if nc.partition_id_tensor else None)
    in_names, out_names, out_avals, zero_outs = [], [], [], []
    for alloc in nc.m.functions[0].allocations:
        if not isinstance(alloc, mybir.MemoryLocationSet):
            continue
        name = alloc.memorylocations[0].name
        if alloc.kind == "ExternalInput":
            if name != partition_name:
                in_names.append(name)
        elif alloc.kind == "ExternalOutput":
            shape = tuple(alloc.tensor_shape)
            dtype = mybir.dt.np(alloc.dtype)
            out_names.append(name)
            out_avals.append(jax.core.ShapedArray(shape, dtype))
            zero_outs.append(np.zeros(shape, dtype))
    n_params = len(in_names)
    n_outs = len(out_avals)
    all_in_names = list(in_names) + list(out_names)
    if partition_name is not None:
        all_in_names.append(partition_name)

    def _body(*args):
        operands = list(args)
        if partition_name is not None:
            operands.append(partition_id_tensor())
        outs = _bass_exec_p.bind(
            *operands,
            out_avals=tuple(out_avals),
            in_names=tuple(all_in_names),
            out_names=tuple(out_names),
            lowering_input_output_aliases=(),
            sim_require_finite=True,
            sim_require_nnan=True,
            nc=nc,
        )
        return tuple(outs)

    devices = jax.devices()[:NCORES]
    mesh = Mesh(np.asarray(devices), ("core",))
    in_specs = (PartitionSpec("core"),) * (n_params + n_outs)
    out_specs = (PartitionSpec("core"),) * n_outs
    sharded = jax.jit(
        shard_map(_body, mesh=mesh, in_specs=in_specs, out_specs=out_specs,
                  check_rep=False),
        donate_argnums=tuple(range(n_params, n_params + n_outs)),
        keep_unused=True,
    )

    def run(in_maps):
        per_core = [[np.asarray(m[nm]) for nm in in_names] for m in in_maps]
        concat_in = [
            np.concatenate([per_core[c][i] for c in range(NCORES)], axis=0)
            for i in range(n_params)
        ]
        concat_zeros = [
            np.zeros((NCORES * z.shape[0], *z.shape[1:]), z.dtype)
            for z in zero_outs
        ]
        out_arrs = sharded(*concat_in, *concat_zeros)
        return [
            {nm: np.asarray(out_arrs[i]).reshape(NCORES, *out_avals[i].shape)[c]
             for i, nm in enumerate(out_names)}
            for c in range(NCORES)
        ]

    return run


def _get_runner():
    global _RUNNER
    if _RUNNER is None:
        nc = _build_program()
        _RUNNER = _make_runner(nc)
    return _RUNNER


def _make_in_maps(x, w_qkv, w_proj, b_proj):
    x = np.ascontiguousarray(np.asarray(x, dtype=np.float32))
    w_qkv = np.ascontiguousarray(np.asarray(w_qkv, dtype=np.float32))
    w_proj = np.ascontiguousarray(np.asarray(w_proj, dtype=np.float32))
    b_proj = np.ascontiguousarray(np.asarray(b_proj, dtype=np.float32))
    in_maps = []
    for c in range(NCORES):
        b, half = divmod(c, 2)
        xc = x[b] if half == 0 else np.concatenate(
            [x[b, NQ:], x[b, :NQ]], axis=0)
        xct = np.ascontiguousarray(xc.T)  # [D, N] for direct lhs/rhs use
        in_maps.append({"xt": xct, "w_qkv": w_qkv, "w_proj": w_proj,
                        "b_proj": b_proj})
    return in_maps


def kernel(x, w_qkv, w_proj, b_proj):
    run = _get_runner()
    results = run(_make_in_maps(x, w_qkv, w_proj, b_proj))
    out = np.empty((B, N, D), dtype=np.float32)
    for c in range(NCORES):
        b, half = divmod(c, 2)
        out[b, half * NQ:(half + 1) * NQ] = results[c]["y"]
    return out



# revision 59
# speedup vs baseline: 1.4664x; 1.4664x over previous
"""Multi-head attention (b=4, n=2048, d=768, h=12) on 8 trn2 NeuronCores.

Sharding: (batch x head-half) -> 8 shards. Each core gets one batch's x and
the qkv/proj weights for 6 of the 12 heads, computes attention for those
heads over the full sequence, and returns the partial projection
y^T_half = W_half^T O_half^T. Host sums the two partials per batch, adds
the bias and transposes. No K/V work is duplicated, no collectives.

Device algorithm (per core):
  1. x^T and the per-half w_qkv ship as fp8e4m3 hi + fp8 residual pairs in
     DoubleRow layout (d = 256c+2p+i interleaved); QKV GEMMs run as three
     fp8 DoubleRow chains (hi*hi + hi*res + res*hi, 256-deep contraction at
     0.5 cycles/row) accumulating in fp32 PSUM -- ~bf16 accuracy at a
     quarter of the bf16 PE cost. w pre-scaled x32 on the host, descaled
     1/32 at the PSUM->SBUF copies (K^T/Q^T bf16; V natural layout with a
     ones column per head for the softmax denominators).
  2. Per head and query-half: S^T[k,q] = K^T_slice.T @ Q^T (bf16);
     P^T = exp(S^T/8) on the ACT engine (25.2M elements/core; ACT is the
     pipeline pacer -- the ISA has no exp on DVE/GPSIMD and GPSIMD cannot
     even read PSUM, so everything else is kept off ACT).
  3. O[q, hd] = sum_kt P^T_kt.T @ V_aug_kt (128 query partitions, 65-wide
     streams; denominator lands in column 64). Normalization is one fused
     scalar_tensor_tensor divide (per-partition scalar read straight from
     the PSUM denominator column) -> O_norm bf16; a PE transpose stacks
     O_norm^T into per-pair [128, 2048] tiles.
  4. Projection woven into the final head: y^T tile = 3 pair matmuls;
     DMAd out as [768, 2048] f32. The bias is added by the host during
     the pair-sum (saves 24 K=1 matmuls in the serial tail).
All emission is software-pipelined: each (head, q-half) S phase weaves in
the previous head's O/normalize/transpose chunks and the next pair's
K/Q/V chunks so the PE never stalls on the exp-paced S-PSUM ring.
TimelineSim: ~244us/core (baseline 394us); CoreSim rel err ~7e-3.
"""

import numpy as np
import ml_dtypes

B, N, D, H, HD = 4, 2048, 768, 12, 64
NQ = 1024              # S/exp tile query width (half the sequence)
SCALE = HD ** -0.5
NCORES = 8
NT = N // 128          # 16 key tiles
QT = NQ // 128         # 8 query blocks per half
NPAIR = 3              # head pairs per core
DHALF = 384            # qkv output dims per core

_RUNNER = None


def _build_program(reps=1):
    import concourse.bass as bass
    import concourse.tile as tile
    import concourse.mybir as mybir
    from concourse import bacc
    from concourse.masks import make_identity
    from contextlib import ExitStack
    from collections import deque

    f32 = mybir.dt.float32
    bf16 = mybir.dt.bfloat16
    f8 = mybir.dt.float8e4
    AF = mybir.ActivationFunctionType
    ALU = mybir.AluOpType
    DR = mybir.MatmulPerfMode.DoubleRow
    W3 = 3 * DHALF     # 1152 qkv out-dims per core

    nc = bacc.Bacc("TRN2", target_bir_lowering=False, debug=False,
                   num_devices=NCORES)

    xdr_in = nc.dram_tensor("xdr", [3 * 128, 2 * N], f8,
                            kind="ExternalInput")
    xrdr_in = nc.dram_tensor("xrdr", [3 * 128, 2 * N], f8,
                             kind="ExternalInput")
    wdr_in = nc.dram_tensor("wdr", [3 * 128, 2 * W3], f8,
                            kind="ExternalInput")
    wrdr_in = nc.dram_tensor("wrdr", [3 * 128, 2 * W3], f8,
                             kind="ExternalInput")
    w_proj = nc.dram_tensor("w_proj", [DHALF, D], bf16,
                            kind="ExternalInput")
    yt = nc.dram_tensor("yt", [D, N], f32, kind="ExternalOutput")

    with tile.TileContext(nc) as tc:
      for _rep in range(reps):
        with ExitStack() as ctx:
          singles = ctx.enter_context(tc.tile_pool(name="singles", bufs=1))
          onat_pool = ctx.enter_context(tc.tile_pool(name="onat", bufs=1))

          identb = singles.tile([128, 128], bf16)
          make_identity(nc, identb)


          # warm the ACT exp table during the startup DMA window
          warm = singles.tile([128, 1], f32)
          nc.vector.memset(warm[:], 0.0)
          nc.scalar.activation(warm[:], warm[:], AF.Exp, bias=0.0, scale=1.0)

          # resident weights: fp8-DR hi/residual row-chunks ([K|Q|V] col
          # order, K+Q DMAd first) + w_proj pair-row blocks
          wdr = [singles.tile([128, 2 * W3], f8, tag=f"wdr{c}",
                              name=f"wdr{c}") for c in range(3)]
          wrdr = [singles.tile([128, 2 * W3], f8, tag=f"wrdr{c}",
                               name=f"wrdr{c}") for c in range(3)]
          wp = [singles.tile([128, D], bf16, tag=f"wp{p}", name=f"wp{p}")
                for p in range(NPAIR)]

          # per-pair O^T tiles (stacked: head A rows 0-63, B 64-127)
          OT = [onat_pool.tile([128, N], bf16, tag=f"ot{p}", name=f"ot{p}")
                for p in range(NPAIR)]

          with tc.tile_pool(name="xt", bufs=1) as xt_pool, \
               tc.tile_pool(name="kq", bufs=2) as kq_pool, \
               tc.tile_pool(name="vsb", bufs=2) as vsb_pool, \
               tc.tile_pool(name="pt", bufs=34) as pt_pool, \
               tc.tile_pool(name="onrm", bufs=10) as onrm_pool, \
               tc.tile_pool(name="rcp", bufs=10) as rcp_pool, \
               tc.tile_pool(name="ysb", bufs=4) as ysb_pool, \
               tc.tile_pool(name="ps_s", bufs=3, space="PSUM") as ps_s, \
               tc.tile_pool(name="ps_misc", bufs=2, space="PSUM") as ps_misc:

              # ---- load x-DR + weights. Transfers serialize per issuing
              # queue (x on sync, w on scalar run in parallel); first
              # n-half of x and the K+Q blocks land first.
              xdr = [xt_pool.tile([128, 2 * N], f8, tag=f"xdr{c}",
                                  name=f"xdr{c}") for c in range(3)]
              xrdr = [xt_pool.tile([128, 2 * N], f8, tag=f"xrdr{c}",
                                   name=f"xrdr{c}") for c in range(3)]
              def seg_dma(eng, dst, srcview, lo, hi, width):
                  eng.dma_start(
                      out=dst[:].rearrange("p (i m) -> p i m",
                                           i=2)[:, :, lo:hi],
                      in_=srcview.rearrange("p (i m) -> p i m",
                                            i=2)[:, :, lo:hi])

              for c in range(3):
                  seg_dma(nc.sync, xdr[c],
                          xdr_in[c * 128:(c + 1) * 128, :], 0, 1024, N)
                  seg_dma(nc.sync, wdr[c],
                          wdr_in[c * 128:(c + 1) * 128, :], 0, 2 * DHALF, W3)
              for c in range(3):
                  seg_dma(nc.sync, xrdr[c],
                          xrdr_in[c * 128:(c + 1) * 128, :], 0, 1024, N)
              for c in range(3):
                  seg_dma(nc.gpsimd, wrdr[c],
                          wrdr_in[c * 128:(c + 1) * 128, :], 0, 2 * DHALF, W3)
              for c in range(3):
                  seg_dma(nc.sync, xdr[c],
                          xdr_in[c * 128:(c + 1) * 128, :], 1024, N, N)
                  seg_dma(nc.sync, xrdr[c],
                          xrdr_in[c * 128:(c + 1) * 128, :], 1024, N, N)
              for c in range(3):
                  seg_dma(nc.gpsimd, wdr[c],
                          wdr_in[c * 128:(c + 1) * 128, :], 2 * DHALF, W3,
                          W3)
                  seg_dma(nc.gpsimd, wrdr[c],
                          wrdr_in[c * 128:(c + 1) * 128, :], 2 * DHALF, W3,
                          W3)
              for p in range(NPAIR):
                  nc.sync.dma_start(out=wp[p][:],
                                    in_=w_proj[p * 128:(p + 1) * 128, :])

              def dr_chain(ps, o0, ow, n0, nw):
                  """9 DoubleRow matmuls: (x@w*32)^T[o0:o0+ow, n0:n0+nw]
                  into ps; hi*hi terms first so startup needs only the hi
                  tensors."""
                  first = True
                  for xa, wa in ((xdr, wdr), (xdr, wrdr), (xrdr, wdr)):
                      for c in range(3):
                          nc.tensor.matmul(
                              ps,
                              lhsT=wa[c][:].rearrange(
                                  "p (i m) -> p i m", i=2)[:, :, o0:o0 + ow],
                              rhs=xa[c][:].rearrange(
                                  "p (i n) -> p i n", i=2)[:, :, n0:n0 + nw],
                              start=first, stop=(c == 2 and xa is xrdr),
                              perf_mode=DR)
                          first = False

              def drv_chain(ps, n0, nw, o0, ow):
                  """Same, natural [n, hd] orientation."""
                  first = True
                  for xa, wa in ((xdr, wdr), (xdr, wrdr), (xrdr, wdr)):
                      for c in range(3):
                          nc.tensor.matmul(
                              ps,
                              lhsT=xa[c][:].rearrange(
                                  "p (i n) -> p i n", i=2)[:, :, n0:n0 + nw],
                              rhs=wa[c][:].rearrange(
                                  "p (i m) -> p i m", i=2)[:, :, o0:o0 + ow],
                              start=first, stop=(c == 2 and xa is xrdr),
                              perf_mode=DR)
                          first = False

              def kq_thunks(p, out):
                  """Thunks: K^T [128, 2048], Q^T [128, 2048] for pair p."""
                  KT = kq_pool.tile([128, N], bf16, tag="kt")
                  QTt = kq_pool.tile([128, N], bf16, tag="qt")
                  out.append((KT, QTt))

                  def chunk(dst, o0, c):
                      ps = ps_misc.tile([128, 512], f32, tag="misc",
                                        name="psk")
                      dr_chain(ps[:], o0, 128, c * 512, 512)
                      nc.vector.tensor_scalar_mul(
                          out=dst[:, c * 512:(c + 1) * 512], in0=ps[:],
                          scalar1=1.0 / 32.0)

                  return ([lambda c=c: chunk(KT, p * 128, c)
                           for c in range(4)]
                          + [lambda c=c: chunk(QTt, DHALF + p * 128, c)
                             for c in range(4)])

              def v_thunks(p, out):
                  """Thunks: V for pair p, natural layout + ones columns."""
                  Vsb = vsb_pool.tile([128, NT, 130], bf16, tag="v")
                  out.append(Vsb)

                  def vhead():
                      nc.gpsimd.memset(Vsb[:, :, 64:65], 1.0)
                      nc.gpsimd.memset(Vsb[:, :, 129:130], 1.0)

                  def vchunk(nt):
                      ps = ps_misc.tile([128, 128], f32, tag="misc",
                                        name="psv")
                      drv_chain(ps[:], nt * 128, 128,
                                2 * DHALF + p * 128, 128)
                      dst = Vsb[:, nt, :].rearrange("n (h c) -> n h c", h=2)
                      nc.vector.tensor_scalar_mul(
                          out=dst[:, :, 0:64],
                          in0=ps[:].rearrange("n (h c) -> n h c", h=2),
                          scalar1=1.0 / 32.0)

                  return [vhead] + [lambda nt=nt: vchunk(nt)
                                    for nt in range(NT)]

              def emit_attn_S(hh, qh, KT, QTt, fill):
                  """S^T + exp for (head hh, q-half qh), weaving `fill`
                  thunks between S tiles so the PE never stalls on the
                  exp-paced S-PSUM ring. Returns P^T tiles."""
                  base = hh * 64
                  q0 = qh * NQ
                  pts = []
                  for _ in range(min(2, len(fill))):
                      fill.popleft()()
                  for kt in range(NT):
                      st = ps_s.tile([128, NQ], f32, tag="st")
                      for i in range(2):
                          nc.tensor.matmul(
                              st[:, i * 512:(i + 1) * 512],
                              lhsT=KT[base:base + 64,
                                      kt * 128:(kt + 1) * 128],
                              rhs=QTt[base:base + 64,
                                      q0 + i * 512:q0 + (i + 1) * 512],
                              start=True, stop=True)
                      pt = pt_pool.tile([128, NQ], bf16, tag="pt")
                      nc.scalar.activation(pt[:], st[:], AF.Exp,
                                           bias=0.0, scale=float(SCALE))
                      pts.append(pt)
                      # keep ACT fed: at most one fill thunk between early
                      # S tiles; back-load the rest where S stalls on the
                      # exp-paced PSUM ring anyway
                      if kt < 8:
                          k = min(1, len(fill))
                      else:
                          k = (len(fill) + NT - 1 - kt) // (NT - kt)
                      for _ in range(k):
                          fill.popleft()()
                  while fill:
                      fill.popleft()()
                  return pts

              def attn_O_thunks(p, hh, qh, Vsb, pts):
                  """Thunks: O chains + normalize, then transposes into
                  the pair-stacked O^T tile."""
                  base = hh * 64
                  q0 = qh * NQ
                  vh = Vsb[:, :, hh * 65: hh * 65 + 65]
                  onrms = []

                  def ochunk(qt):
                      po = ps_misc.tile([128, 65], f32, tag="misc",
                                        name="po")
                      for kt in range(NT):
                          nc.tensor.matmul(
                              po[:],
                              lhsT=pts[kt][:, qt * 128:(qt + 1) * 128],
                              rhs=vh[:, kt, :],
                              start=(kt == 0), stop=(kt == NT - 1))
                      rcp = rcp_pool.tile([128, 1], f32, tag="rc")
                      nc.vector.reciprocal(rcp[:], po[:, 64:65])
                      onrm = onrm_pool.tile([128, 64], bf16, tag="on")
                      nc.vector.tensor_scalar_mul(
                          out=onrm[:], in0=po[:, 0:64], scalar1=rcp[:])
                      onrms.append(onrm)

                  def trchunk(qt):
                      tr = ps_misc.tile([128, 128], bf16, tag="misc",
                                        name="ptr")
                      nc.tensor.transpose(
                          tr[base:base + 64, :], onrms[qt][:], identb[:])
                      nc.vector.tensor_copy(
                          out=OT[p][base:base + 64,
                                    q0 + qt * 128:q0 + (qt + 1) * 128],
                          in_=tr[base:base + 64, :])

                  return ([lambda qt=qt: ochunk(qt) for qt in range(QT)]
                          + [lambda qt=qt: trchunk(qt) for qt in range(QT)])

              def proj_thunks():
                  """Thunk groups by q-chunk: y^T tile = bias-seed +
                  sum_p W_p^T O_p^T, copy + DMA out."""
                  def ptile(do, qc):
                      pp = ps_misc.tile([128, 512], f32, tag="misc",
                                        name="pp")
                      for p in range(NPAIR):
                          nc.tensor.matmul(
                              pp[:],
                              lhsT=wp[p][:, do * 128:(do + 1) * 128],
                              rhs=OT[p][:, qc * 512:(qc + 1) * 512],
                              start=(p == 0), stop=(p == NPAIR - 1))
                      ys = ysb_pool.tile([128, 512], f32, tag="ys")
                      if qc >= 2:   # tail: ACT is idle by then
                          nc.scalar.activation(ys[:], pp[:], AF.Copy,
                                               bias=0.0, scale=1.0)
                      else:
                          nc.vector.tensor_copy(out=ys[:], in_=pp[:])
                      nc.sync.dma_start(
                          out=yt[do * 128:(do + 1) * 128,
                                 qc * 512:(qc + 1) * 512],
                          in_=ys[:])

                  return [[lambda do=do, qc=qc: ptile(do, qc)
                           for do in range(6)] for qc in range(4)]

              # ---- software pipeline over 12 virtual heads
              # (pair, head, q-half); each S phase absorbs the previous
              # vhead's O chunks + the next pair's K/Q/V chunks; the
              # last heads also absorb the projection tiles.
              handles = []
              kq0 = kq_thunks(0, handles)
              for t in (kq0[0], kq0[4], kq0[5]):   # K c0, Q c0, Q c1
                  t()
              carry = deque(kq0[1:4] + kq0[6:8])
              carry.extend(v_thunks(0, handles))
              KT, QTt = handles[0]
              Vsb = handles[1]
              cur = (KT, QTt, Vsb)
              pending = []
              pj = proj_thunks()
              # last pair runs (hh, qh) transposed so the first q-half of
              # BOTH its heads finalizes two vheads early -- the 12 early
              # projection tiles then spread 6+6 instead of all landing in
              # the final vhead's S phase
              vheads = ([(p, hh, qh) for p in range(NPAIR - 1)
                         for hh in range(2) for qh in range(2)]
                        + [(2, 0, 0), (2, 1, 0), (2, 0, 1), (2, 1, 1)])
              for iv, (p, hh, qh) in enumerate(vheads):
                  if p + 1 < NPAIR:
                      extra = []
                      if (hh, qh) == (0, 1):
                          carry.extend(kq_thunks(p + 1, extra))
                          nkq = extra[0]
                      elif (hh, qh) == (1, 0):
                          carry.extend(v_thunks(p + 1, extra))
                          nvs = extra[0]
                  if iv == 10:
                      carry.extend(pj[0])
                  elif iv == 11:
                      carry.extend(pj[1])
                  fill = deque(pending)
                  # pair-0 prologue must fully land before its O phase
                  take = len(carry) if iv == 0 else min(len(carry), 7)
                  for _ in range(take):
                      fill.append(carry.popleft())
                  pts = emit_attn_S(hh, qh, cur[0], cur[1], fill)
                  pending = attn_O_thunks(p, hh, qh, cur[2], pts)
                  if p + 1 < NPAIR and (hh, qh) == (1, 1):
                      cur = (nkq[0], nkq[1], nvs)
              # tail: final vhead's O phase with the last proj tiles woven
              while carry:
                  carry.popleft()()
              tail = (pending[0:4] + pending[8:12] + pj[2]
                      + pending[4:8] + pending[12:16] + pj[3])
              for t in tail:
                  t()

    nc.compile()
    return nc


def _make_runner(nc):
    """Cached multi-core PJRT runner (jitted callable built once)."""
    import jax
    from jax.experimental.shard_map import shard_map
    from jax.sharding import Mesh, PartitionSpec
    import concourse.mybir as mybir
    from concourse.bass2jax import (_bass_exec_p, install_neuronx_cc_hook,
                                    partition_id_tensor)

    install_neuronx_cc_hook()

    partition_name = (nc.partition_id_tensor.name
                      if nc.partition_id_tensor else None)
    in_names, out_names, out_avals, zero_outs = [], [], [], []
    for alloc in nc.m.functions[0].allocations:
        if not isinstance(alloc, mybir.MemoryLocationSet):
            continue
        name = alloc.memorylocations[0].name
        if alloc.kind == "ExternalInput":
            if name != partition_name:
                in_names.append(name)
        elif alloc.kind == "ExternalOutput":
            shape = tuple(alloc.tensor_shape)
            dtype = mybir.dt.np(alloc.dtype)
            out_names.append(name)
            out_avals.append(jax.core.ShapedArray(shape, dtype))
            zero_outs.append(np.zeros(shape, dtype))
    n_params = len(in_names)
    all_in_names = list(in_names) + list(out_names)
    if partition_name is not None:
        all_in_names.append(partition_name)

    def _body(*args):
        operands = list(args)
        if partition_name is not None:
            operands.append(partition_id_tensor())
        outs = _bass_exec_p.bind(
            *operands,
            out_avals=tuple(out_avals),
            in_names=tuple(all_in_names),
            out_names=tuple(out_names),
            lowering_input_output_aliases=(),
            sim_require_finite=True,
            sim_require_nnan=True,
            nc=nc,
        )
        return tuple(outs)

    devices = jax.devices()[:NCORES]
    mesh = Mesh(np.asarray(devices), ("core",))
    in_specs = (PartitionSpec("core"),) * (n_params + len(out_avals))
    out_specs = (PartitionSpec("core"),) * len(out_avals)
    sharded = jax.jit(
        shard_map(_body, mesh=mesh, in_specs=in_specs, out_specs=out_specs,
                  check_rep=False),
        donate_argnums=tuple(range(n_params, n_params + len(out_avals))),
        keep_unused=True,
    )

    def run(in_maps):
        per_core = [[np.asarray(m[nm]) for nm in in_names] for m in in_maps]
        concat_in = [
            np.concatenate([per_core[c][i] for c in range(NCORES)], axis=0)
            for i in range(n_params)
        ]
        concat_zeros = [
            np.zeros((NCORES * z.shape[0], *z.shape[1:]), z.dtype)
            for z in zero_outs
        ]
        out_arrs = sharded(*concat_in, *concat_zeros)
        return [
            {nm: np.asarray(out_arrs[i]).reshape(NCORES, *out_avals[i].shape)[c]
             for i, nm in enumerate(out_names)}
            for c in range(NCORES)
        ]

    return run


def _get_runner():
    global _RUNNER
    if _RUNNER is None:
        nc = _build_program()
        _RUNNER = _make_runner(nc)
    return _RUNNER


def _dr_split(a):
    """[768, C] f32 -> (hi, res) fp8e4m3 in DoubleRow layout [384, 2C]:
    row 128c+p, col 2o+i holds a[256c+2p+i, o]."""
    f8 = ml_dtypes.float8_e4m3fn
    hi = a.astype(f8)
    res = (a - hi.astype(np.float32)).astype(f8)
    out = []
    for m in (hi, res):
        # (c, p, i, o) flattened: row 128c+p, col i*C + o (segmented halves)
        out.append(np.ascontiguousarray(m.reshape(384, 2 * a.shape[1])))
    return out


def _make_in_maps(x, w_qkv, w_proj, b_proj):
    bf = ml_dtypes.bfloat16
    x = np.asarray(x, dtype=np.float32)
    wq = np.asarray(w_qkv, dtype=np.float32)
    wpj = np.asarray(w_proj, dtype=np.float32)
    # per-batch x in DoubleRow hi/res (shared by the two head-half cores)
    xparts = [_dr_split(np.ascontiguousarray(x[b].T)) for b in range(B)]
    # per-half weights: [K|Q|V] col order, x32 scale
    wparts, wpparts = [], []
    for hf in range(2):
        s = slice(hf * DHALF, (hf + 1) * DHALF)
        wq_p = np.concatenate(
            [wq[:, D:2 * D][:, s], wq[:, 0:D][:, s], wq[:, 2 * D:][:, s]],
            axis=1) * np.float32(32)
        wparts.append(_dr_split(wq_p))
        wpparts.append(np.ascontiguousarray(wpj[s, :]).astype(bf))
    in_maps = []
    for c in range(NCORES):
        b, hf = divmod(c, 2)
        in_maps.append({
            "xdr": xparts[b][0], "xrdr": xparts[b][1],
            "wdr": wparts[hf][0], "wrdr": wparts[hf][1],
            "w_proj": wpparts[hf],
        })
    return in_maps


def kernel(x, w_qkv, w_proj, b_proj):
    run = _get_runner()
    results = run(_make_in_maps(x, w_qkv, w_proj, b_proj))
    bias = np.asarray(b_proj, dtype=np.float32)[None, :]
    out = np.empty((B, N, D), dtype=np.float32)
    for b in range(B):
        out[b] = (results[2 * b]["yt"] + results[2 * b + 1]["yt"]).T + bias
    return out


# revision 69
# speedup vs baseline: 1.4703x; 1.0027x over previous
"""Multi-head attention (b=4, n=2048, d=768, h=12) on 8 trn2 NeuronCores.

Sharding: (batch x head-half) -> 8 shards. Each core gets one batch's x and
the qkv/proj weights for 6 of the 12 heads, computes attention for those
heads over the full sequence, and returns the partial projection
y^T_half = W_half^T O_half^T. Host sums the two partials per batch, adds
the bias and transposes. No K/V work is duplicated, no collectives.

Device algorithm (per core):
  1. x^T and the per-half w_qkv ship as fp8e4m3 hi + fp8 residual pairs in
     DoubleRow layout (d = 256c+2p+i interleaved); QKV GEMMs run as three
     fp8 DoubleRow chains (hi*hi + hi*res + res*hi, 256-deep contraction at
     0.5 cycles/row) accumulating in fp32 PSUM -- ~bf16 accuracy at a
     quarter of the bf16 PE cost. w pre-scaled x32 on the host, descaled
     1/32 at the PSUM->SBUF copies (K^T/Q^T bf16; V natural layout with a
     ones column per head for the softmax denominators).
  2. Per head and query-half: S^T[k,q] = K^T_slice.T @ Q^T (bf16);
     P^T = exp(S^T/8) on the ACT engine (25.2M elements/core; ACT is the
     pipeline pacer -- the ISA has no exp on DVE/GPSIMD and GPSIMD cannot
     even read PSUM, so everything else is kept off ACT).
  3. O[q, hd] = sum_kt P^T_kt.T @ V_aug_kt (128 query partitions, 65-wide
     streams; denominator lands in column 64). Normalization is one fused
     scalar_tensor_tensor divide (per-partition scalar read straight from
     the PSUM denominator column) -> O_norm bf16; a PE transpose stacks
     O_norm^T into per-pair [128, 2048] tiles.
  4. Projection woven into the final head: y^T tile = 3 pair matmuls;
     DMAd out as [768, 2048] f32. The bias is added by the host during
     the pair-sum (saves 24 K=1 matmuls in the serial tail).
All emission is software-pipelined: each (head, q-half) S phase weaves in
the previous head's O/normalize/transpose chunks and the next pair's
K/Q/V chunks so the PE never stalls on the exp-paced S-PSUM ring.
TimelineSim: ~239.9us/core (baseline 394us); CoreSim rel err ~7e-3.
"""

import numpy as np
import ml_dtypes

B, N, D, H, HD = 4, 2048, 768, 12, 64
NQ = 1024              # S/exp tile query width (half the sequence)
SCALE = HD ** -0.5
NCORES = 8
NT = N // 128          # 16 key tiles
QT = NQ // 128         # 8 query blocks per half
NPAIR = 3              # head pairs per core
DHALF = 384            # qkv output dims per core

_RUNNER = None


def _build_program(reps=1):
    import concourse.bass as bass
    import concourse.tile as tile
    import concourse.mybir as mybir
    from concourse import bacc
    from concourse.masks import make_identity
    from contextlib import ExitStack
    from collections import deque

    f32 = mybir.dt.float32
    bf16 = mybir.dt.bfloat16
    f8 = mybir.dt.float8e4
    AF = mybir.ActivationFunctionType
    ALU = mybir.AluOpType
    DR = mybir.MatmulPerfMode.DoubleRow
    W3 = 3 * DHALF     # 1152 qkv out-dims per core

    nc = bacc.Bacc("TRN2", target_bir_lowering=False, debug=False,
                   num_devices=NCORES)

    xdr_in = nc.dram_tensor("xdr", [3 * 128, 2 * N], f8,
                            kind="ExternalInput")
    xrdr_in = nc.dram_tensor("xrdr", [3 * 128, 2 * N], f8,
                             kind="ExternalInput")
    wdr_in = nc.dram_tensor("wdr", [3 * 128, 2 * W3], f8,
                            kind="ExternalInput")
    wrdr_in = nc.dram_tensor("wrdr", [3 * 128, 2 * W3], f8,
                             kind="ExternalInput")
    w_proj = nc.dram_tensor("w_proj", [DHALF, D], bf16,
                            kind="ExternalInput")
    yt = nc.dram_tensor("yt", [D, N], f32, kind="ExternalOutput")

    with tile.TileContext(nc) as tc:
      for _rep in range(reps):
        with ExitStack() as ctx:
          singles = ctx.enter_context(tc.tile_pool(name="singles", bufs=1))
          onat_pool = ctx.enter_context(tc.tile_pool(name="onat", bufs=1))

          identb = singles.tile([128, 128], bf16)
          make_identity(nc, identb)


          # warm the ACT exp table during the startup DMA window
          warm = singles.tile([128, 1], f32)
          nc.vector.memset(warm[:], 0.0)
          nc.scalar.activation(warm[:], warm[:], AF.Exp, bias=0.0, scale=1.0)

          # resident weights: fp8-DR hi/residual row-chunks ([K|Q|V] col
          # order, K+Q DMAd first) + w_proj pair-row blocks
          wdr = [singles.tile([128, 2 * W3], f8, tag=f"wdr{c}",
                              name=f"wdr{c}") for c in range(3)]
          wrdr = [singles.tile([128, 2 * W3], f8, tag=f"wrdr{c}",
                               name=f"wrdr{c}") for c in range(3)]
          wp = [singles.tile([128, D], bf16, tag=f"wp{p}", name=f"wp{p}")
                for p in range(NPAIR)]

          # per-pair O^T tiles (stacked: head A rows 0-63, B 64-127)
          OT = [onat_pool.tile([128, N], bf16, tag=f"ot{p}", name=f"ot{p}")
                for p in range(NPAIR)]

          with tc.tile_pool(name="xt", bufs=1) as xt_pool, \
               tc.tile_pool(name="kq", bufs=2) as kq_pool, \
               tc.tile_pool(name="vsb", bufs=2) as vsb_pool, \
               tc.tile_pool(name="pt", bufs=34) as pt_pool, \
               tc.tile_pool(name="onrm", bufs=10) as onrm_pool, \
               tc.tile_pool(name="rcp", bufs=10) as rcp_pool, \
               tc.tile_pool(name="ysb", bufs=4) as ysb_pool, \
               tc.tile_pool(name="ps_s", bufs=3, space="PSUM") as ps_s, \
               tc.tile_pool(name="ps_misc", bufs=2, space="PSUM") as ps_misc:

              # ---- load x-DR + weights. Transfers serialize per issuing
              # queue (x on sync, w on scalar run in parallel); first
              # n-half of x and the K+Q blocks land first.
              xdr = [xt_pool.tile([128, 2 * N], f8, tag=f"xdr{c}",
                                  name=f"xdr{c}") for c in range(3)]
              xrdr = [xt_pool.tile([128, 2 * N], f8, tag=f"xrdr{c}",
                                   name=f"xrdr{c}") for c in range(3)]
              def seg_dma(eng, dst, srcview, lo, hi, width):
                  eng.dma_start(
                      out=dst[:].rearrange("p (i m) -> p i m",
                                           i=2)[:, :, lo:hi],
                      in_=srcview.rearrange("p (i m) -> p i m",
                                            i=2)[:, :, lo:hi])

              for c in range(3):
                  seg_dma(nc.sync, xdr[c],
                          xdr_in[c * 128:(c + 1) * 128, :], 0, 1024, N)
                  seg_dma(nc.sync, wdr[c],
                          wdr_in[c * 128:(c + 1) * 128, :], 0, 2 * DHALF, W3)
              for c in range(3):
                  seg_dma(nc.sync, xrdr[c],
                          xrdr_in[c * 128:(c + 1) * 128, :], 0, 1024, N)
              for c in range(3):
                  seg_dma(nc.gpsimd, wrdr[c],
                          wrdr_in[c * 128:(c + 1) * 128, :], 0, 2 * DHALF, W3)
              for c in range(3):
                  seg_dma(nc.sync, xdr[c],
                          xdr_in[c * 128:(c + 1) * 128, :], 1024, N, N)
                  seg_dma(nc.sync, xrdr[c],
                          xrdr_in[c * 128:(c + 1) * 128, :], 1024, N, N)
              for c in range(3):
                  seg_dma(nc.gpsimd, wdr[c],
                          wdr_in[c * 128:(c + 1) * 128, :], 2 * DHALF, W3,
                          W3)
                  seg_dma(nc.gpsimd, wrdr[c],
                          wrdr_in[c * 128:(c + 1) * 128, :], 2 * DHALF, W3,
                          W3)
              for p in range(NPAIR):
                  nc.sync.dma_start(out=wp[p][:],
                                    in_=w_proj[p * 128:(p + 1) * 128, :])

              def dr_chain(ps, o0, ow, n0, nw):
                  """9 DoubleRow matmuls: (x@w*32)^T[o0:o0+ow, n0:n0+nw]
                  into ps; hi*hi terms first so startup needs only the hi
                  tensors."""
                  first = True
                  for xa, wa in ((xdr, wdr), (xdr, wrdr), (xrdr, wdr)):
                      for c in range(3):
                          nc.tensor.matmul(
                              ps,
                              lhsT=wa[c][:].rearrange(
                                  "p (i m) -> p i m", i=2)[:, :, o0:o0 + ow],
                              rhs=xa[c][:].rearrange(
                                  "p (i n) -> p i n", i=2)[:, :, n0:n0 + nw],
                              start=first, stop=(c == 2 and xa is xrdr),
                              perf_mode=DR)
                          first = False

              def drv_chain(ps, n0, nw, o0, ow):
                  """Same, natural [n, hd] orientation."""
                  first = True
                  for xa, wa in ((xdr, wdr), (xdr, wrdr), (xrdr, wdr)):
                      for c in range(3):
                          nc.tensor.matmul(
                              ps,
                              lhsT=xa[c][:].rearrange(
                                  "p (i n) -> p i n", i=2)[:, :, n0:n0 + nw],
                              rhs=wa[c][:].rearrange(
                                  "p (i m) -> p i m", i=2)[:, :, o0:o0 + ow],
                              start=first, stop=(c == 2 and xa is xrdr),
                              perf_mode=DR)
                          first = False

              def kq_thunks(p, out):
                  """Thunks: K^T [128, 2048], Q^T [128, 2048] for pair p."""
                  KT = kq_pool.tile([128, N], bf16, tag="kt")
                  QTt = kq_pool.tile([128, N], bf16, tag="qt")
                  out.append((KT, QTt))

                  def chunk(dst, o0, c):
                      ps = ps_misc.tile([128, 512], f32, tag="misc",
                                        name="psk")
                      dr_chain(ps[:], o0, 128, c * 512, 512)
                      nc.vector.tensor_scalar_mul(
                          out=dst[:, c * 512:(c + 1) * 512], in0=ps[:],
                          scalar1=1.0 / 32.0)

                  return ([lambda c=c: chunk(KT, p * 128, c)
                           for c in range(4)]
                          + [lambda c=c: chunk(QTt, DHALF + p * 128, c)
                             for c in range(4)])

              def v_thunks(p, out):
                  """Thunks: V for pair p, natural layout + ones columns."""
                  Vsb = vsb_pool.tile([128, NT, 130], bf16, tag="v")
                  out.append(Vsb)

                  def vhead():
                      nc.gpsimd.memset(Vsb[:, :, 64:65], 1.0)
                      nc.gpsimd.memset(Vsb[:, :, 129:130], 1.0)

                  def vchunk(nt):
                      ps = ps_misc.tile([128, 128], f32, tag="misc",
                                        name="psv")
                      drv_chain(ps[:], nt * 128, 128,
                                2 * DHALF + p * 128, 128)
                      dst = Vsb[:, nt, :].rearrange("n (h c) -> n h c", h=2)
                      nc.vector.tensor_scalar_mul(
                          out=dst[:, :, 0:64],
                          in0=ps[:].rearrange("n (h c) -> n h c", h=2),
                          scalar1=1.0 / 32.0)

                  return [vhead] + [lambda nt=nt: vchunk(nt)
                                    for nt in range(NT)]

              def emit_attn_S(hh, qh, KT, QTt, fill):
                  """S^T + exp for (head hh, q-half qh), weaving `fill`
                  thunks between S tiles so the PE never stalls on the
                  exp-paced S-PSUM ring. Returns P^T tiles."""
                  base = hh * 64
                  q0 = qh * NQ
                  pts = []
                  for _ in range(min(2, len(fill))):
                      fill.popleft()()
                  for kt in range(NT):
                      st = ps_s.tile([128, NQ], f32, tag="st")
                      for i in range(2):
                          nc.tensor.matmul(
                              st[:, i * 512:(i + 1) * 512],
                              lhsT=KT[base:base + 64,
                                      kt * 128:(kt + 1) * 128],
                              rhs=QTt[base:base + 64,
                                      q0 + i * 512:q0 + (i + 1) * 512],
                              start=True, stop=True)
                      pt = pt_pool.tile([128, NQ], bf16, tag="pt")
                      nc.scalar.activation(pt[:], st[:], AF.Exp,
                                           bias=0.0, scale=float(SCALE))
                      pts.append(pt)
                      # keep ACT fed: at most one fill thunk between early
                      # S tiles; back-load the rest where S stalls on the
                      # exp-paced PSUM ring anyway
                      if kt < 8:
                          k = min(1, len(fill))
                      else:
                          k = (len(fill) + NT - 1 - kt) // (NT - kt)
                      for _ in range(k):
                          fill.popleft()()
                  while fill:
                      fill.popleft()()
                  return pts

              def attn_O_thunks(p, hh, qh, Vsb, pts):
                  """Thunks: O chains + normalize, then transposes into
                  the pair-stacked O^T tile."""
                  base = hh * 64
                  q0 = qh * NQ
                  vh = Vsb[:, :, hh * 65: hh * 65 + 65]
                  onrms = []

                  def ochunk(qt):
                      po = ps_misc.tile([128, 65], f32, tag="misc",
                                        name="po")
                      for kt in range(NT):
                          nc.tensor.matmul(
                              po[:],
                              lhsT=pts[kt][:, qt * 128:(qt + 1) * 128],
                              rhs=vh[:, kt, :],
                              start=(kt == 0), stop=(kt == NT - 1))
                      rcp = rcp_pool.tile([128, 1], f32, tag="rc")
                      nc.vector.reciprocal(rcp[:], po[:, 64:65])
                      onrm = onrm_pool.tile([128, 64], bf16, tag="on")
                      nc.vector.tensor_scalar_mul(
                          out=onrm[:], in0=po[:, 0:64], scalar1=rcp[:])
                      onrms.append(onrm)

                  def trchunk(qt):
                      tr = ps_misc.tile([128, 128], bf16, tag="misc",
                                        name="ptr")
                      nc.tensor.transpose(
                          tr[base:base + 64, :], onrms[qt][:], identb[:])
                      nc.vector.tensor_copy(
                          out=OT[p][base:base + 64,
                                    q0 + qt * 128:q0 + (qt + 1) * 128],
                          in_=tr[base:base + 64, :])

                  return ([lambda qt=qt: ochunk(qt) for qt in range(QT)]
                          + [lambda qt=qt: trchunk(qt) for qt in range(QT)])

              def proj_thunks():
                  """Thunk groups by q-chunk: y^T tile = bias-seed +
                  sum_p W_p^T O_p^T, copy + DMA out."""
                  def ptile(do, qc):
                      pp = ps_misc.tile([128, 512], f32, tag="misc",
                                        name="pp")
                      for p in range(NPAIR):
                          nc.tensor.matmul(
                              pp[:],
                              lhsT=wp[p][:, do * 128:(do + 1) * 128],
                              rhs=OT[p][:, qc * 512:(qc + 1) * 512],
                              start=(p == 0), stop=(p == NPAIR - 1))
                      ys = ysb_pool.tile([128, 512], f32, tag="ys")
                      if qc >= 2:   # tail: ACT is idle by then
                          nc.scalar.activation(ys[:], pp[:], AF.Copy,
                                               bias=0.0, scale=1.0)
                      else:
                          nc.vector.tensor_copy(out=ys[:], in_=pp[:])
                      nc.sync.dma_start(
                          out=yt[do * 128:(do + 1) * 128,
                                 qc * 512:(qc + 1) * 512],
                          in_=ys[:])

                  return [[lambda do=do, qc=qc: ptile(do, qc)
                           for do in range(6)] for qc in range(4)]

              # ---- software pipeline over 12 virtual heads
              # (pair, head, q-half); each S phase absorbs the previous
              # vhead's O chunks + the next pair's K/Q/V chunks; the
              # last heads also absorb the projection tiles.
              handles = []
              kq0 = kq_thunks(0, handles)
              # prologue: K c0 + Q c0/c1 with all hi-terms emitted before
              # any residual terms -- the residual DMAs land ~3us later,
              # so the hi matmuls overlap the DMA window. Uses S-pool
              # slots (free this early) for the 3 concurrent groups.
              KT0, QT0 = handles[0]
              pro = [(KT0, 0, 0), (QT0, DHALF, 0), (QT0, DHALF, 512)]
              pps = []
              for i in range(3):
                  pps.append(ps_s.tile([128, 512], f32, tag="st",
                                       name=f"pro{i}"))
              for xa, wa in ((xdr, wdr), (xdr, wrdr), (xrdr, wdr)):
                  for i, (dst, o0, n0) in enumerate(pro):
                      for c in range(3):
                          nc.tensor.matmul(
                              pps[i][:],
                              lhsT=wa[c][:].rearrange(
                                  "p (i m) -> p i m",
                                  i=2)[:, :, o0:o0 + 128],
                              rhs=xa[c][:].rearrange(
                                  "p (i n) -> p i n",
                                  i=2)[:, :, n0:n0 + 512],
                              start=(xa is xdr and wa is wdr and c == 0),
                              stop=(xa is xrdr and c == 2),
                              perf_mode=DR)
              for i, (dst, o0, n0) in enumerate(pro):
                  nc.vector.tensor_scalar_mul(
                      out=dst[:, n0:n0 + 512], in0=pps[i][:],
                      scalar1=1.0 / 32.0)
              carry = deque(kq0[1:4] + kq0[6:8])
              carry.extend(v_thunks(0, handles))
              KT, QTt = handles[0]
              Vsb = handles[1]
              cur = (KT, QTt, Vsb)
              pending = []
              pj = proj_thunks()
              # last pair runs (hh, qh) transposed so the first q-half of
              # BOTH its heads finalizes two vheads early -- the 12 early
              # projection tiles then spread 6+6 instead of all landing in
              # the final vhead's S phase
              vheads = ([(p, hh, qh) for p in range(NPAIR - 1)
                         for hh in range(2) for qh in range(2)]
                        + [(2, 0, 0), (2, 1, 0), (2, 0, 1), (2, 1, 1)])
              for iv, (p, hh, qh) in enumerate(vheads):
                  if p + 1 < NPAIR:
                      extra = []
                      if (hh, qh) == (0, 1):
                          carry.extend(kq_thunks(p + 1, extra))
                          nkq = extra[0]
                      elif (hh, qh) == (1, 0):
                          carry.extend(v_thunks(p + 1, extra))
                          nvs = extra[0]
                  if iv == 10:
                      carry.extend(pj[0])
                  elif iv == 11:
                      carry.extend(pj[1])
                  fill = deque(pending)
                  # pair-0 prologue must fully land before its O phase
                  take = len(carry) if iv == 0 else min(len(carry), 7)
                  for _ in range(take):
                      fill.append(carry.popleft())
                  pts = emit_attn_S(hh, qh, cur[0], cur[1], fill)
                  pending = attn_O_thunks(p, hh, qh, cur[2], pts)
                  if p + 1 < NPAIR and (hh, qh) == (1, 1):
                      cur = (nkq[0], nkq[1], nvs)
              # tail: final vhead's O phase with the last proj tiles woven
              while carry:
                  carry.popleft()()
              tail = (pending[0:4] + pending[8:12] + pj[2]
                      + pending[4:8] + pending[12:16] + pj[3])
              for t in tail:
                  t()

    nc.compile()
    return nc


def _make_runner(nc):
    """Cached multi-core PJRT runner (jitted callable built once)."""
    import jax
    from jax.experimental.shard_map import shard_map
    from jax.sharding import Mesh, PartitionSpec
    import concourse.mybir as mybir
    from concourse.bass2jax import (_bass_exec_p, install_neuronx_cc_hook,
                                    partition_id_tensor)

    install_neuronx_cc_hook()

    partition_name = (nc.partition_id_tensor.name
                      if nc.partition_id_tensor else None)
    in_names, out_names, out_avals, zero_outs = [], [], [], []
    for alloc in nc.m.functions[0].allocations:
        if not isinstance(alloc, mybir.MemoryLocationSet):
            continue
        name = alloc.memorylocations[0].name
        if alloc.kind == "ExternalInput":
            if name != partition_name:
                in_names.append(name)
        elif alloc.kind == "ExternalOutput":
            shape = tuple(alloc.tensor_shape)
            dtype = mybir.dt.np(alloc.dtype)
            out_names.append(name)
            out_avals.append(jax.core.ShapedArray(shape, dtype))
            zero_outs.append(np.zeros(shape, dtype))
    n_params = len(in_names)
    all_in_names = list(in_names) + list(out_names)
    if partition_name is not None:
        all_in_names.append(partition_name)

    def _body(*args):
        operands = list(args)
        if partition_name is not None:
            operands.append(partition_id_tensor())
        outs = _bass_exec_p.bind(
            *operands,
            out_avals=tuple(out_avals),
            in_names=tuple(all_in_names),
            out_names=tuple(out_names),
            lowering_input_output_aliases=(),
            sim_require_finite=True,
            sim_require_nnan=True,
            nc=nc,
        )
        return tuple(outs)

    devices = jax.devices()[:NCORES]
    mesh = Mesh(np.asarray(devices), ("core",))
    in_specs = (PartitionSpec("core"),) * (n_params + len(out_avals))
    out_specs = (PartitionSpec("core"),) * len(out_avals)
    sharded = jax.jit(
        shard_map(_body, mesh=mesh, in_specs=in_specs, out_specs=out_specs,
                  check_rep=False),
        donate_argnums=tuple(range(n_params, n_params + len(out_avals))),
        keep_unused=True,
    )

    def run(in_maps):
        per_core = [[np.asarray(m[nm]) for nm in in_names] for m in in_maps]
        concat_in = [
            np.concatenate([per_core[c][i] for c in range(NCORES)], axis=0)
            for i in range(n_params)
        ]
        concat_zeros = [
            np.zeros((NCORES * z.shape[0], *z.shape[1:]), z.dtype)
            for z in zero_outs
        ]
        out_arrs = sharded(*concat_in, *concat_zeros)
        return [
            {nm: np.asarray(out_arrs[i]).reshape(NCORES, *out_avals[i].shape)[c]
             for i, nm in enumerate(out_names)}
            for c in range(NCORES)
        ]

    return run


def _get_runner():
    global _RUNNER
    if _RUNNER is None:
        nc = _build_program()
        _RUNNER = _make_runner(nc)
    return _RUNNER


def _dr_split(a):
    """[768, C] f32 -> (hi, res) fp8e4m3 in DoubleRow layout [384, 2C]:
    row 128c+p, col 2o+i holds a[256c+2p+i, o]."""
    f8 = ml_dtypes.float8_e4m3fn
    hi = a.astype(f8)
    res = (a - hi.astype(np.float32)).astype(f8)
    out = []
    for m in (hi, res):
        # (c, p, i, o) flattened: row 128c+p, col i*C + o (segmented halves)
        out.append(np.ascontiguousarray(m.reshape(384, 2 * a.shape[1])))
    return out


def _make_in_maps(x, w_qkv, w_proj, b_proj):
    bf = ml_dtypes.bfloat16
    x = np.asarray(x, dtype=np.float32)
    wq = np.asarray(w_qkv, dtype=np.float32)
    wpj = np.asarray(w_proj, dtype=np.float32)
    # per-batch x in DoubleRow hi/res (shared by the two head-half cores)
    xparts = [_dr_split(np.ascontiguousarray(x[b].T)) for b in range(B)]
    # per-half weights: [K|Q|V] col order, x32 scale
    wparts, wpparts = [], []
    for hf in range(2):
        s = slice(hf * DHALF, (hf + 1) * DHALF)
        wq_p = np.concatenate(
            [wq[:, D:2 * D][:, s], wq[:, 0:D][:, s], wq[:, 2 * D:][:, s]],
            axis=1) * np.float32(32)
        wparts.append(_dr_split(wq_p))
        wpparts.append(np.ascontiguousarray(wpj[s, :]).astype(bf))
    in_maps = []
    for c in range(NCORES):
        b, hf = divmod(c, 2)
        in_maps.append({
            "xdr": xparts[b][0], "xrdr": xparts[b][1],
            "wdr": wparts[hf][0], "wrdr": wparts[hf][1],
            "w_proj": wpparts[hf],
        })
    return in_maps


def kernel(x, w_qkv, w_proj, b_proj):
    run = _get_runner()
    results = run(_make_in_maps(x, w_qkv, w_proj, b_proj))
    bias = np.asarray(b_proj, dtype=np.float32)[None, :]
    out = np.empty((B, N, D), dtype=np.float32)
    for b in range(B):
        out[b] = (results[2 * b]["yt"] + results[2 * b + 1]["yt"]).T + bias
    return out


# revision 70
# speedup vs baseline: 1.4822x; 1.0081x over previous
"""Multi-head attention (b=4, n=2048, d=768, h=12) on 8 trn2 NeuronCores.

Sharding: (batch x head-half) -> 8 shards. Each core gets one batch's x and
the qkv/proj weights for 6 of the 12 heads, computes attention for those
heads over the full sequence, and returns the partial projection
y^T_half = W_half^T O_half^T. Host sums the two partials per batch, adds
the bias and transposes. No K/V work is duplicated, no collectives.

Device algorithm (per core):
  1. x^T and the per-half w_qkv ship as fp8e4m3 hi + fp8 residual pairs in
     DoubleRow layout (d = 256c+2p+i interleaved); QKV GEMMs run as three
     fp8 DoubleRow chains (hi*hi + hi*res + res*hi, 256-deep contraction at
     0.5 cycles/row) accumulating in fp32 PSUM -- ~bf16 accuracy at a
     quarter of the bf16 PE cost. w pre-scaled x32 on the host, descaled
     1/32 at the PSUM->SBUF copies (K^T/Q^T bf16; V natural layout with a
     ones column per head for the softmax denominators).
  2. Per head and query-half: S^T[k,q] = K^T_slice.T @ Q^T (bf16);
     P^T = exp(S^T/8) on the ACT engine (25.2M elements/core; ACT is the
     pipeline pacer -- the ISA has no exp on DVE/GPSIMD and GPSIMD cannot
     even read PSUM, so everything else is kept off ACT).
  3. O[q, hd] = sum_kt P^T_kt.T @ V_aug_kt (128 query partitions, 65-wide
     streams; denominator lands in column 64). Normalization is one fused
     scalar_tensor_tensor divide (per-partition scalar read straight from
     the PSUM denominator column) -> O_norm bf16; a PE transpose stacks
     O_norm^T into per-pair [128, 2048] tiles.
  4. Projection woven into the final head: y^T tile = 3 pair matmuls;
     DMAd out as [768, 2048] f32. The bias is added by the host during
     the pair-sum (saves 24 K=1 matmuls in the serial tail).
All emission is software-pipelined: each (head, q-half) S phase weaves in
the previous head's O/normalize/transpose chunks and the next pair's
K/Q/V chunks so the PE never stalls on the exp-paced S-PSUM ring.
TimelineSim: ~239.9us/core (baseline 394us); CoreSim rel err ~7e-3.
"""

import numpy as np
import ml_dtypes

B, N, D, H, HD = 4, 2048, 768, 12, 64
NQ = 1024              # S/exp tile query width (half the sequence)
SCALE = HD ** -0.5
NCORES = 8
NT = N // 128          # 16 key tiles
QT = NQ // 128         # 8 query blocks per half
NPAIR = 3              # head pairs per core
DHALF = 384            # qkv output dims per core

_RUNNER = None


def _build_program(reps=1):
    import concourse.bass as bass
    import concourse.tile as tile
    import concourse.mybir as mybir
    from concourse import bacc
    from concourse.masks import make_identity
    from contextlib import ExitStack
    from collections import deque

    f32 = mybir.dt.float32
    bf16 = mybir.dt.bfloat16
    f8 = mybir.dt.float8e4
    AF = mybir.ActivationFunctionType
    ALU = mybir.AluOpType
    DR = mybir.MatmulPerfMode.DoubleRow
    W3 = 3 * DHALF     # 1152 qkv out-dims per core

    nc = bacc.Bacc("TRN2", target_bir_lowering=False, debug=False,
                   num_devices=NCORES)

    xdr_in = nc.dram_tensor("xdr", [3 * 128, 2 * N], f8,
                            kind="ExternalInput")
    xrdr_in = nc.dram_tensor("xrdr", [3 * 128, 2 * N], f8,
                             kind="ExternalInput")
    wdr_in = nc.dram_tensor("wdr", [3 * 128, 2 * W3], f8,
                            kind="ExternalInput")
    wrdr_in = nc.dram_tensor("wrdr", [3 * 128, 2 * W3], f8,
                             kind="ExternalInput")
    w_proj = nc.dram_tensor("w_proj", [DHALF, D], bf16,
                            kind="ExternalInput")
    yt = nc.dram_tensor("yt", [D, N], f32, kind="ExternalOutput")

    with tile.TileContext(nc) as tc:
      for _rep in range(reps):
        with ExitStack() as ctx:
          singles = ctx.enter_context(tc.tile_pool(name="singles", bufs=1))
          onat_pool = ctx.enter_context(tc.tile_pool(name="onat", bufs=1))

          identb = singles.tile([128, 128], bf16)
          make_identity(nc, identb)


          # warm the ACT exp table during the startup DMA window
          warm = singles.tile([128, 1], f32)
          nc.vector.memset(warm[:], 0.0)
          nc.scalar.activation(warm[:], warm[:], AF.Exp, bias=0.0, scale=1.0)

          # resident weights: fp8-DR hi/residual row-chunks ([K|Q|V] col
          # order, K+Q DMAd first) + w_proj pair-row blocks
          wdr = [singles.tile([128, 2 * W3], f8, tag=f"wdr{c}",
                              name=f"wdr{c}") for c in range(3)]
          wrdr = [singles.tile([128, 2 * W3], f8, tag=f"wrdr{c}",
                               name=f"wrdr{c}") for c in range(3)]
          wp = [singles.tile([128, D], bf16, tag=f"wp{p}", name=f"wp{p}")
                for p in range(NPAIR)]

          # per-pair O^T tiles (stacked: head A rows 0-63, B 64-127)
          OT = [onat_pool.tile([128, N], bf16, tag=f"ot{p}", name=f"ot{p}")
                for p in range(NPAIR)]

          with tc.tile_pool(name="xt", bufs=1) as xt_pool, \
               tc.tile_pool(name="kq", bufs=2) as kq_pool, \
               tc.tile_pool(name="vsb", bufs=2) as vsb_pool, \
               tc.tile_pool(name="pt", bufs=34) as pt_pool, \
               tc.tile_pool(name="onrm", bufs=10) as onrm_pool, \
               tc.tile_pool(name="rcp", bufs=10) as rcp_pool, \
               tc.tile_pool(name="ysb", bufs=4) as ysb_pool, \
               tc.tile_pool(name="ps_s", bufs=3, space="PSUM") as ps_s, \
               tc.tile_pool(name="ps_misc", bufs=2, space="PSUM") as ps_misc:

              # ---- load x-DR + weights. Transfers serialize per issuing
              # queue (x on sync, w on scalar run in parallel); first
              # n-half of x and the K+Q blocks land first.
              xdr = [xt_pool.tile([128, 2 * N], f8, tag=f"xdr{c}",
                                  name=f"xdr{c}") for c in range(3)]
              xrdr = [xt_pool.tile([128, 2 * N], f8, tag=f"xrdr{c}",
                                   name=f"xrdr{c}") for c in range(3)]
              def seg_dma(eng, dst, srcview, lo, hi, width):
                  eng.dma_start(
                      out=dst[:].rearrange("p (i m) -> p i m",
                                           i=2)[:, :, lo:hi],
                      in_=srcview.rearrange("p (i m) -> p i m",
                                            i=2)[:, :, lo:hi])

              for c in range(3):
                  seg_dma(nc.sync, xdr[c],
                          xdr_in[c * 128:(c + 1) * 128, :], 0, 1024, N)
                  seg_dma(nc.sync, wdr[c],
                          wdr_in[c * 128:(c + 1) * 128, :], 0, 2 * DHALF, W3)
              for c in range(3):
                  seg_dma(nc.sync, xrdr[c],
                          xrdr_in[c * 128:(c + 1) * 128, :], 0, 1024, N)
              for c in range(3):
                  seg_dma(nc.gpsimd, wrdr[c],
                          wrdr_in[c * 128:(c + 1) * 128, :], 0, 2 * DHALF, W3)
              for c in range(3):
                  seg_dma(nc.sync, xdr[c],
                          xdr_in[c * 128:(c + 1) * 128, :], 1024, N, N)
                  seg_dma(nc.sync, xrdr[c],
                          xrdr_in[c * 128:(c + 1) * 128, :], 1024, N, N)
              for c in range(3):
                  seg_dma(nc.gpsimd, wdr[c],
                          wdr_in[c * 128:(c + 1) * 128, :], 2 * DHALF, W3,
                          W3)
                  seg_dma(nc.gpsimd, wrdr[c],
                          wrdr_in[c * 128:(c + 1) * 128, :], 2 * DHALF, W3,
                          W3)
              for p in range(NPAIR):
                  nc.sync.dma_start(out=wp[p][:],
                                    in_=w_proj[p * 128:(p + 1) * 128, :])

              def dr_chain(ps, o0, ow, n0, nw):
                  """9 DoubleRow matmuls: (x@w*32)^T[o0:o0+ow, n0:n0+nw]
                  into ps; hi*hi terms first so startup needs only the hi
                  tensors."""
                  first = True
                  for xa, wa in ((xdr, wdr), (xdr, wrdr), (xrdr, wdr)):
                      for c in range(3):
                          nc.tensor.matmul(
                              ps,
                              lhsT=wa[c][:].rearrange(
                                  "p (i m) -> p i m", i=2)[:, :, o0:o0 + ow],
                              rhs=xa[c][:].rearrange(
                                  "p (i n) -> p i n", i=2)[:, :, n0:n0 + nw],
                              start=first, stop=(c == 2 and xa is xrdr),
                              perf_mode=DR)
                          first = False

              def drv_chain(ps, n0, nw, o0, ow):
                  """Same, natural [n, hd] orientation."""
                  first = True
                  for xa, wa in ((xdr, wdr), (xdr, wrdr), (xrdr, wdr)):
                      for c in range(3):
                          nc.tensor.matmul(
                              ps,
                              lhsT=xa[c][:].rearrange(
                                  "p (i n) -> p i n", i=2)[:, :, n0:n0 + nw],
                              rhs=wa[c][:].rearrange(
                                  "p (i m) -> p i m", i=2)[:, :, o0:o0 + ow],
                              start=first, stop=(c == 2 and xa is xrdr),
                              perf_mode=DR)
                          first = False

              def kq_thunks(p, out):
                  """Thunks: K^T [128, 2048], Q^T [128, 2048] for pair p."""
                  KT = kq_pool.tile([128, N], bf16, tag="kt")
                  QTt = kq_pool.tile([128, N], bf16, tag="qt")
                  out.append((KT, QTt))

                  def chunk(dst, o0, c):
                      ps = ps_misc.tile([128, 512], f32, tag="misc",
                                        name="psk")
                      dr_chain(ps[:], o0, 128, c * 512, 512)
                      nc.vector.tensor_scalar_mul(
                          out=dst[:, c * 512:(c + 1) * 512], in0=ps[:],
                          scalar1=1.0 / 32.0)

                  return ([lambda c=c: chunk(KT, p * 128, c)
                           for c in range(4)]
                          + [lambda c=c: chunk(QTt, DHALF + p * 128, c)
                             for c in range(4)])

              def v_thunks(p, out):
                  """Thunks: V for pair p, natural layout + ones columns."""
                  Vsb = vsb_pool.tile([128, NT, 130], bf16, tag="v")
                  out.append(Vsb)

                  def vhead():
                      nc.gpsimd.memset(Vsb[:, :, 64:65], 1.0)
                      nc.gpsimd.memset(Vsb[:, :, 129:130], 1.0)

                  def vchunk(nt):
                      ps = ps_misc.tile([128, 128], f32, tag="misc",
                                        name="psv")
                      drv_chain(ps[:], nt * 128, 128,
                                2 * DHALF + p * 128, 128)
                      dst = Vsb[:, nt, :].rearrange("n (h c) -> n h c", h=2)
                      nc.vector.tensor_scalar_mul(
                          out=dst[:, :, 0:64],
                          in0=ps[:].rearrange("n (h c) -> n h c", h=2),
                          scalar1=1.0 / 32.0)

                  return [vhead] + [lambda nt=nt: vchunk(nt)
                                    for nt in range(NT)]

              def emit_attn_S(hh, qh, KT, QTt, fill):
                  """S^T + exp for (head hh, q-half qh), weaving `fill`
                  thunks between S tiles so the PE never stalls on the
                  exp-paced S-PSUM ring. Returns P^T tiles."""
                  base = hh * 64
                  q0 = qh * NQ
                  pts = []
                  for _ in range(min(2, len(fill))):
                      fill.popleft()()
                  for kt in range(NT):
                      st = ps_s.tile([128, NQ], f32, tag="st")
                      for i in range(2):
                          nc.tensor.matmul(
                              st[:, i * 512:(i + 1) * 512],
                              lhsT=KT[base:base + 64,
                                      kt * 128:(kt + 1) * 128],
                              rhs=QTt[base:base + 64,
                                      q0 + i * 512:q0 + (i + 1) * 512],
                              start=True, stop=True)
                      pt = pt_pool.tile([128, NQ], bf16, tag="pt")
                      nc.scalar.activation(pt[:], st[:], AF.Exp,
                                           bias=0.0, scale=float(SCALE))
                      pts.append(pt)
                      # keep ACT fed: at most one fill thunk between early
                      # S tiles; back-load the rest where S stalls on the
                      # exp-paced PSUM ring anyway
                      if kt < 8:
                          k = min(1, len(fill))
                      else:
                          k = (len(fill) + NT - 1 - kt) // (NT - kt)
                      for _ in range(k):
                          fill.popleft()()
                  while fill:
                      fill.popleft()()
                  return pts

              def attn_O_thunks(p, hh, qh, Vsb, pts):
                  """Thunks: O chains + normalize, then transposes into
                  the pair-stacked O^T tile."""
                  base = hh * 64
                  q0 = qh * NQ
                  vh = Vsb[:, :, hh * 65: hh * 65 + 65]
                  onrms = []

                  def ochunk(qt):
                      po = ps_misc.tile([128, 65], f32, tag="misc",
                                        name="po")
                      for kt in range(NT):
                          nc.tensor.matmul(
                              po[:],
                              lhsT=pts[kt][:, qt * 128:(qt + 1) * 128],
                              rhs=vh[:, kt, :],
                              start=(kt == 0), stop=(kt == NT - 1))
                      rcp = rcp_pool.tile([128, 1], f32, tag="rc")
                      nc.vector.reciprocal(rcp[:], po[:, 64:65])
                      onrm = onrm_pool.tile([128, 64], bf16, tag="on")
                      nc.vector.tensor_scalar_mul(
                          out=onrm[:], in0=po[:, 0:64], scalar1=rcp[:])
                      onrms.append(onrm)

                  def trchunk(qt):
                      tr = ps_misc.tile([128, 128], bf16, tag="misc",
                                        name="ptr")
                      nc.tensor.transpose(
                          tr[base:base + 64, :], onrms[qt][:], identb[:])
                      nc.vector.tensor_copy(
                          out=OT[p][base:base + 64,
                                    q0 + qt * 128:q0 + (qt + 1) * 128],
                          in_=tr[base:base + 64, :])

                  return ([lambda qt=qt: ochunk(qt) for qt in range(QT)]
                          + [lambda qt=qt: trchunk(qt) for qt in range(QT)])

              def proj_thunks():
                  """Thunk groups by q-chunk: y^T tile = bias-seed +
                  sum_p W_p^T O_p^T, copy + DMA out."""
                  def ptile(do, qc):
                      pp = ps_misc.tile([128, 512], f32, tag="misc",
                                        name="pp")
                      for p in range(NPAIR):
                          nc.tensor.matmul(
                              pp[:],
                              lhsT=wp[p][:, do * 128:(do + 1) * 128],
                              rhs=OT[p][:, qc * 512:(qc + 1) * 512],
                              start=(p == 0), stop=(p == NPAIR - 1))
                      ys = ysb_pool.tile([128, 512], f32, tag="ys")
                      if qc >= 2:   # tail: ACT is idle by then
                          nc.scalar.activation(ys[:], pp[:], AF.Copy,
                                               bias=0.0, scale=1.0)
                      else:
                          nc.vector.tensor_copy(out=ys[:], in_=pp[:])
                      nc.sync.dma_start(
                          out=yt[do * 128:(do + 1) * 128,
                                 qc * 512:(qc + 1) * 512],
                          in_=ys[:])

                  return [[lambda do=do, qc=qc: ptile(do, qc)
                           for do in range(6)] for qc in range(4)]

              # ---- software pipeline over 12 virtual heads
              # (pair, head, q-half); each S phase absorbs the previous
              # vhead's O chunks + the next pair's K/Q/V chunks; the
              # last heads also absorb the projection tiles.
              handles = []
              kq0 = kq_thunks(0, handles)
              # prologue: K c0 + Q c0/c1 with all hi-terms emitted before
              # any residual terms -- the residual DMAs land ~3us later,
              # so the hi matmuls overlap the DMA window. Uses S-pool
              # slots (free this early) for the 3 concurrent groups.
              KT0, QT0 = handles[0]
              pro = [(KT0, 0, 0), (QT0, DHALF, 0), (QT0, DHALF, 512)]
              pps = []
              for i in range(3):
                  pps.append(ps_s.tile([128, 512], f32, tag="st",
                                       name=f"pro{i}"))
              for xa, wa in ((xdr, wdr), (xdr, wrdr), (xrdr, wdr)):
                  for i, (dst, o0, n0) in enumerate(pro):
                      for c in range(3):
                          nc.tensor.matmul(
                              pps[i][:],
                              lhsT=wa[c][:].rearrange(
                                  "p (i m) -> p i m",
                                  i=2)[:, :, o0:o0 + 128],
                              rhs=xa[c][:].rearrange(
                                  "p (i n) -> p i n",
                                  i=2)[:, :, n0:n0 + 512],
                              start=(xa is xdr and wa is wdr and c == 0),
                              stop=(xa is xrdr and c == 2),
                              perf_mode=DR)
              for i, (dst, o0, n0) in enumerate(pro):
                  nc.vector.tensor_scalar_mul(
                      out=dst[:, n0:n0 + 512], in0=pps[i][:],
                      scalar1=1.0 / 32.0)
              carry = deque(kq0[1:4] + kq0[6:8])
              carry.extend(v_thunks(0, handles))
              KT, QTt = handles[0]
              Vsb = handles[1]
              cur = (KT, QTt, Vsb)
              pending = []
              pj = proj_thunks()
              # last pair runs (hh, qh) transposed so the first q-half of
              # BOTH its heads finalizes two vheads early -- the 12 early
              # projection tiles then spread 6+6 instead of all landing in
              # the final vhead's S phase
              vheads = ([(p, hh, qh) for p in range(NPAIR - 1)
                         for hh in range(2) for qh in range(2)]
                        + [(2, 0, 0), (2, 1, 0), (2, 0, 1), (2, 1, 1)])
              for iv, (p, hh, qh) in enumerate(vheads):
                  if p + 1 < NPAIR:
                      extra = []
                      if (hh, qh) == (0, 1):
                          carry.extend(kq_thunks(p + 1, extra))
                          nkq = extra[0]
                      elif (hh, qh) == (1, 0):
                          carry.extend(v_thunks(p + 1, extra))
                          nvs = extra[0]
                  if iv == 10:
                      carry.extend(pj[0])
                  elif iv == 11:
                      carry.extend(pj[1])
                  fill = deque(pending)
                  # pair-0 prologue must fully land before its O phase
                  take = (len(carry) if iv % 4 == 0
                          else min(len(carry), 6))
                  for _ in range(take):
                      fill.append(carry.popleft())
                  pts = emit_attn_S(hh, qh, cur[0], cur[1], fill)
                  pending = attn_O_thunks(p, hh, qh, cur[2], pts)
                  if p + 1 < NPAIR and (hh, qh) == (1, 1):
                      cur = (nkq[0], nkq[1], nvs)
              # tail: final vhead's O phase with the last proj tiles woven
              while carry:
                  carry.popleft()()
              tail = (pending[0:4] + pending[8:12] + pj[2]
                      + pending[4:8] + pending[12:16] + pj[3])
              for t in tail:
                  t()

    nc.compile()
    return nc


def _make_runner(nc):
    """Cached multi-core PJRT runner (jitted callable built once)."""
    import jax
    from jax.experimental.shard_map import shard_map
    from jax.sharding import Mesh, PartitionSpec
    import concourse.mybir as mybir
    from concourse.bass2jax import (_bass_exec_p, install_neuronx_cc_hook,
                                    partition_id_tensor)

    install_neuronx_cc_hook()

    partition_name = (nc.partition_id_tensor.name
                      if nc.partition_id_tensor else None)
    in_names, out_names, out_avals, zero_outs = [], [], [], []
    for alloc in nc.m.functions[0].allocations:
        if not isinstance(alloc, mybir.MemoryLocationSet):
            continue
        name = alloc.memorylocations[0].name
        if alloc.kind == "ExternalInput":
            if name != partition_name:
                in_names.append(name)
        elif alloc.kind == "ExternalOutput":
            shape = tuple(alloc.tensor_shape)
            dtype = mybir.dt.np(alloc.dtype)
            out_names.append(name)
            out_avals.append(jax.core.ShapedArray(shape, dtype))
            zero_outs.append(np.zeros(shape, dtype))
    n_params = len(in_names)
    all_in_names = list(in_names) + list(out_names)
    if partition_name is not None:
        all_in_names.append(partition_name)

    def _body(*args):
        operands = list(args)
        if partition_name is not None:
            operands.append(partition_id_tensor())
        outs = _bass_exec_p.bind(
            *operands,
            out_avals=tuple(out_avals),
            in_names=tuple(all_in_names),
            out_names=tuple(out_names),
            lowering_input_output_aliases=(),
            sim_require_finite=True,
            sim_require_nnan=True,
            nc=nc,
        )
        return tuple(outs)

    devices = jax.devices()[:NCORES]
    mesh = Mesh(np.asarray(devices), ("core",))
    in_specs = (PartitionSpec("core"),) * (n_params + len(out_avals))
    out_specs = (PartitionSpec("core"),) * len(out_avals)
    sharded = jax.jit(
        shard_map(_body, mesh=mesh, in_specs=in_specs, out_specs=out_specs,
                  check_rep=False),
        donate_argnums=tuple(range(n_params, n_params + len(out_avals))),
        keep_unused=True,
    )

    def run(in_maps):
        per_core = [[np.asarray(m[nm]) for nm in in_names] for m in in_maps]
        concat_in = [
            np.concatenate([per_core[c][i] for c in range(NCORES)], axis=0)
            for i in range(n_params)
        ]
        concat_zeros = [
            np.zeros((NCORES * z.shape[0], *z.shape[1:]), z.dtype)
            for z in zero_outs
        ]
        out_arrs = sharded(*concat_in, *concat_zeros)
        return [
            {nm: np.asarray(out_arrs[i]).reshape(NCORES, *out_avals[i].shape)[c]
             for i, nm in enumerate(out_names)}
            for c in range(NCORES)
        ]

    return run


def _get_runner():
    global _RUNNER
    if _RUNNER is None:
        nc = _build_program()
        _RUNNER = _make_runner(nc)
    return _RUNNER


def _dr_split(a):
    """[768, C] f32 -> (hi, res) fp8e4m3 in DoubleRow layout [384, 2C]:
    row 128c+p, col 2o+i holds a[256c+2p+i, o]."""
    f8 = ml_dtypes.float8_e4m3fn
    hi = a.astype(f8)
    res = (a - hi.astype(np.float32)).astype(f8)
    out = []
    for m in (hi, res):
        # (c, p, i, o) flattened: row 128c+p, col i*C + o (segmented halves)
        out.append(np.ascontiguousarray(m.reshape(384, 2 * a.shape[1])))
    return out


def _make_in_maps(x, w_qkv, w_proj, b_proj):
    bf = ml_dtypes.bfloat16
    x = np.asarray(x, dtype=np.float32)
    wq = np.asarray(w_qkv, dtype=np.float32)
    wpj = np.asarray(w_proj, dtype=np.float32)
    # per-batch x in DoubleRow hi/res (shared by the two head-half cores)
    xparts = [_dr_split(np.ascontiguousarray(x[b].T)) for b in range(B)]
    # per-half weights: [K|Q|V] col order, x32 scale
    wparts, wpparts = [], []
    for hf in range(2):
        s = slice(hf * DHALF, (hf + 1) * DHALF)
        wq_p = np.concatenate(
            [wq[:, D:2 * D][:, s], wq[:, 0:D][:, s], wq[:, 2 * D:][:, s]],
            axis=1) * np.float32(32)
        wparts.append(_dr_split(wq_p))
        wpparts.append(np.ascontiguousarray(wpj[s, :]).astype(bf))
    in_maps = []
    for c in range(NCORES):
        b, hf = divmod(c, 2)
        in_maps.append({
            "xdr": xparts[b][0], "xrdr": xparts[b][1],
            "wdr": wparts[hf][0], "wrdr": wparts[hf][1],
            "w_proj": wpparts[hf],
        })
    return in_maps


def kernel(x, w_qkv, w_proj, b_proj):
    run = _get_runner()
    results = run(_make_in_maps(x, w_qkv, w_proj, b_proj))
    bias = np.asarray(b_proj, dtype=np.float32)[None, :]
    out = np.empty((B, N, D), dtype=np.float32)
    for b in range(B):
        out[b] = (results[2 * b]["yt"] + results[2 * b + 1]["yt"]).T + bias
    return out
